# revision 1
# baseline (speedup 1.0000x reference)
"""Causal attention (single head) on 8 Trainium2 NeuronCores.

Problem: x[4096,1024], Wq/Wk/Wv[1024,1024] (torch Linear layout, applied as
x @ W.T); out = renormalized-causal-softmax(Q K^T / 32) @ V, fp32, [4096,1024].

Distribution (hardcoded for S=4096, D=1024, 8 cores):
  - Q rows are sharded STRIDED: core c owns rows c::8, so every core has
    identical causal trip counts (SPMD: one program, data-only variation).
  - K/V rows are sharded CONTIGUOUS: core c computes K^T,V for rows
    [512c, 512c+512) in fp8e3 (e3m4: 4-bit mantissa; K/V values sit well
    inside its range, and it halves the exchange bytes vs bf16/e4m3).
  - Measured CC behavior on this runtime: the collective stream moves no
    bytes until ~75us after EXECUTION start (per-exec arming, independent
    of trigger time, op count, or payload), then streams at ~80GB/s wire
    with ~8us/op overhead.  Two collectives -- AG(KT, 2MB out) then
    AG(V, 4MB out) -- measured fastest; more splits lower the wire rate.
  - Attention is computed TRANSPOSED: S^T[k, q] = K^T-chunk.T @ Q^T with the
    128-key chunk stationary.  exp(S^T) IS P^T, exactly the lhsT layout the
    P@V matmuls need -- no PE transposes.  Causality: key chunk kc is needed
    only by q >= 16*kc (exact, uniformly over cores); within that only the
    first 16 q columns are partially masked, by ONE shared [128,16] additive
    -30000 pattern (keep iff c+8u >= j, core-dependent data).  The
    causally-dead strip [128*(kc//8), 16*kc) that P@V still touches is
    zeroed.  "softmax -> tril -> renormalize" == masked-exp / masked-sum,
    and scores/32 are within +-4 so exp needs no max-subtraction.
  - Denominators: P@V runs with P^T chunks stationary; a third 1-column
    matmul against ones rides on each stationary load and accumulates
    sum_k P^T[k, q] in PSUM -- the softmax denominator for free.
  - All matmuls bf16/fp8 mixed (fp32 matmul is 4x slower); PSUM accum fp32.
    (Measured dead end: filling the CC-wait PE gaps with dummy matmuls to
    keep the HAM clock gate warm made things WORSE -- the chip runs under a
    GPIO power cap, so extra PE work directly lowers the duty clock.)
"""

import numpy as np
import ml_dtypes

S, D, NC_N = 4096, 1024, 8
QROWS = S // NC_N            # 512 q rows per core
KVROWS = S // NC_N           # 512 kv rows per core
NQT = QROWS // 128           # 4 q-tiles of 128 rows per core
DC = D // 128                # 8 contraction chunks
NKC = S // 128               # 32 global key chunks
BF16 = ml_dtypes.bfloat16

_CACHE = {}


def _build():
    import concourse.bass as bass
    import concourse.mybir as mybir
    import concourse.tile as tile
    from concourse import bacc

    fp32 = mybir.dt.float32
    bf16 = mybir.dt.bfloat16
    fp8 = mybir.dt.float8e3

    nc = bacc.Bacc("TRN2", target_bir_lowering=False, debug=False,
                   num_devices=NC_N, enable_asserts=False)

    xt_q = nc.dram_tensor("xt_q", [D, QROWS], bf16, kind="ExternalInput").ap()
    xt_kv = nc.dram_tensor("xt_kv", [D, KVROWS], bf16, kind="ExternalInput").ap()
    wqt = nc.dram_tensor("wqt", [D, D], bf16, kind="ExternalInput").ap()
    wkt = nc.dram_tensor("wkt", [D, D], bf16, kind="ExternalInput").ap()
    wvt = nc.dram_tensor("wvt", [D, D], bf16, kind="ExternalInput").ap()
    maskT = nc.dram_tensor("maskT", [128, 16], fp32, kind="ExternalInput").ap()
    out = nc.dram_tensor("out", [QROWS, D], fp32, kind="ExternalOutput").ap()

    rg = [list(range(NC_N))]
    inv_sqrt_d = 1.0 / np.sqrt(np.float32(D))

    with tile.TileContext(nc) as tc:
        with (
            tc.tile_pool(name="dram", bufs=1, space="DRAM") as dram,
            tc.tile_pool(name="const", bufs=1) as cpool,
            tc.tile_pool(name="kvres", bufs=1) as kvpool,
            tc.tile_pool(name="stats", bufs=4) as stpool,
        ):
            kt_cc_in = dram.tile([D, KVROWS], fp8, name="kt_cc_in")
            v_cc_in = dram.tile([KVROWS, D], fp8, name="v_cc_in")
            kt_cc_out = dram.tile([NC_N, D, KVROWS], fp8, name="kt_cc_out",
                                  addr_space="Shared")
            v_cc_out = dram.tile([NC_N, KVROWS, D], fp8, name="v_cc_out",
                                 addr_space="Shared")

            ones_sb = cpool.tile([128, 1], bf16, name="ones_sb")
            nc.any.memset(ones_sb[:], 1.0)
            mask_sb = cpool.tile([128, 16], fp32, name="mask_sb")
            nc.scalar.dma_start(mask_sb[:], maskT[:])
            wrm = cpool.tile([128, 512], bf16, name="wrm")
            nc.vector.memset(wrm[:], 0.0)

            # gathered K^T, 2 shards per tile:
            #   ktq[h][p, (r-2h)*4096 + dc*512 + j] = K[512r+j, 128dc+p]
            ktq = [kvpool.tile([128, 2 * DC * 512], fp8, name=f"ktq{h}")
                   for h in range(4)]
            # gathered V, 2 shards per tile:
            #   vfp[g][p, (r-2g)*4096 + sl*1024 + dh*512 + j]
            #     = V[512r+128sl+p, 512dh+j]
            vfp = [kvpool.tile([128, 2 * 4096], fp8, name=f"vfp{g}")
                   for g in range(4)]
            # Q^T resident: qt_sb[p, dc*512 + q] = Q[q, 128dc+p]
            qt_sb = kvpool.tile([128, DC * 512], bf16, name="qt_sb")
            # P^T resident: ptall[p, kc*512 + q] = P[q, 128kc+p]
            ptall = kvpool.tile([128, NKC * 512], bf16, name="ptall")

            # ---------------- phase 1: projections + gathers ----------------
            with (
                tc.tile_pool(name="wpool", bufs=12) as wpool,
                tc.tile_pool(name="xpool", bufs=1) as xpool,
                tc.tile_pool(name="loc", bufs=4) as locpool,
                tc.tile_pool(name="ppsum", bufs=3, space="PSUM") as ppsum,
            ):
                # PE warmup: the HAM clock gate holds the PE at 1.2 GHz until
                # it has been busy ~3.4us; burn dummy matmuls during the
                # initial weight DMA so the projections run warm (measured
                # better than omitting it, despite the GPIO power cap).
                wps = ppsum.tile([128, 512], fp32, tag="warm")
                for i in range(24):
                    nc.tensor.matmul(wps[:], wrm[:, 0:128], wrm[:],
                                     start=(i == 0), stop=(i == 23))

                # K-projection inputs first (critical path to the CC stream)
                wk, xkv = [], []
                for dc in range(DC):
                    tw = wpool.tile([128, D], bf16, name=f"wk{dc}", tag="w")
                    nc.scalar.dma_start(tw[:], wkt[dc * 128:(dc + 1) * 128, :])
                    wk.append(tw)
                    tx = xpool.tile([128, KVROWS], bf16, name=f"xkv{dc}")
                    nc.scalar.dma_start(tx[:], xt_kv[dc * 128:(dc + 1) * 128, :])
                    xkv.append(tx)

                # K^T_local[d, s] = (Wk @ x_kv^T): lhsT = Wk^T chunk, rhs = x_kv^T
                for po in range(DC):
                    ps = ppsum.tile([128, 512], fp32, tag="pp")
                    for dc in range(DC):
                        nc.tensor.matmul(ps[:], wk[dc][:, po * 128:(po + 1) * 128],
                                         xkv[dc][:],
                                         start=(dc == 0), stop=(dc == DC - 1))
                    loc = locpool.tile([128, 512], fp8, tag="lock")
                    nc.vector.tensor_copy(loc[:], ps[:])
                    nc.sync.dma_start(kt_cc_in[po * 128:(po + 1) * 128, :], loc[:])

                nc.gpsimd.collective_compute(
                    "AllGather", mybir.AluOpType.bypass, replica_groups=rg,
                    ins=[kt_cc_in[:]], outs=[kt_cc_out[:]])

                # V_local[s, d] = x_kv @ Wv^T: lhsT = x_kv^T chunk, rhs = Wv^T
                wv = []
                for dc in range(DC):
                    tw = wpool.tile([128, D], bf16, name=f"wv{dc}", tag="w")
                    nc.scalar.dma_start(tw[:], wvt[dc * 128:(dc + 1) * 128, :])
                    wv.append(tw)
                for st in range(4):
                    for dh in range(2):
                        ps = ppsum.tile([128, 512], fp32, tag="pp")
                        for dc in range(DC):
                            nc.tensor.matmul(
                                ps[:], xkv[dc][:, st * 128:(st + 1) * 128],
                                wv[dc][:, dh * 512:(dh + 1) * 512],
                                start=(dc == 0), stop=(dc == DC - 1))
                        loc = locpool.tile([128, 512], fp8, tag="locv")
                        nc.vector.tensor_copy(loc[:], ps[:])
                        nc.sync.dma_start(
                            v_cc_in[st * 128:(st + 1) * 128,
                                    dh * 512:(dh + 1) * 512], loc[:])
                nc.gpsimd.collective_compute(
                    "AllGather", mybir.AluOpType.bypass, replica_groups=rg,
                    ins=[v_cc_in[:]], outs=[v_cc_out[:]])

                # Q^T[d, q]: lhsT = Wq^T chunk, rhs = x_q^T  -> straight to SBUF
                wq, xq = [], []
                for dc in range(DC):
                    tw = wpool.tile([128, D], bf16, name=f"wq{dc}", tag="w")
                    nc.scalar.dma_start(tw[:], wqt[dc * 128:(dc + 1) * 128, :])
                    wq.append(tw)
                    tx = xpool.tile([128, QROWS], bf16, name=f"xq{dc}")
                    nc.scalar.dma_start(tx[:], xt_q[dc * 128:(dc + 1) * 128, :])
                    xq.append(tx)
                for po in range(DC):
                    ps = ppsum.tile([128, 512], fp32, tag="pp")
                    for dc in range(DC):
                        nc.tensor.matmul(ps[:], wq[dc][:, po * 128:(po + 1) * 128],
                                         xq[dc][:],
                                         start=(dc == 0), stop=(dc == DC - 1))
                    nc.vector.tensor_copy(qt_sb[:, po * 512:(po + 1) * 512], ps[:])

            # ---------------- phase 2: pull gathered K/V into SBUF ----------
            # Few, fat DMAs: the pulls are descriptor/fixed-latency bound, so
            # 2 K pulls + 4 V pulls (1KB descriptors, both d-halves of a V
            # row in one run) instead of 24 small ones.  Per-group tiles let
            # P@V on q-tile qt gate only on the V groups it actually reads.
            for h in range(4):
                eng = nc.sync if h % 2 == 0 else nc.scalar
                eng.dma_start(
                    ktq[h][:].rearrange("p (r a j) -> p r a j", r=2, a=DC),
                    kt_cc_out[2 * h:2 * h + 2].rearrange(
                        "r (a p) j -> p r a j", p=128))
            for g in range(4):
                eng = nc.sync if g % 2 == 0 else nc.scalar
                eng.dma_start(
                    vfp[g][:].rearrange("p (r s q) -> p r s q", r=2, s=4),
                    v_cc_out[2 * g:2 * g + 2].rearrange(
                        "r (s p) q -> p r s q", p=128))

            # ---------------- phase 3a: S^T + exp -> P^T ----------------
            with tc.tile_pool(name="spsum", bufs=6, space="PSUM") as spsum:
                for kc in range(NKC):
                    qt0 = kc // 8
                    qoff = 16 * kc
                    w = 512 - qoff
                    r, kci = kc // 4, kc % 4
                    if qoff > 128 * qt0:
                        nc.vector.memset(
                            ptall[:, kc * 512 + 128 * qt0:kc * 512 + qoff], 0.0)
                    psT = spsum.tile([128, 512], fp32, tag="s")
                    rb = (r % 2) * 4096
                    for dc in range(DC):
                        nc.tensor.matmul(
                            psT[:, :w],
                            ktq[r // 2][:, rb + dc * 512 + kci * 128:
                                        rb + dc * 512 + (kci + 1) * 128],
                            qt_sb[:, dc * 512 + qoff:(dc + 1) * 512],
                            start=(dc == 0), stop=(dc == DC - 1))
                    nc.vector.tensor_add(psT[:, 0:16], psT[:, 0:16],
                                         mask_sb[:])
                    nc.scalar.activation(
                        ptall[:, kc * 512 + qoff:(kc + 1) * 512], psT[:, :w],
                        mybir.ActivationFunctionType.Exp,
                        bias=0.0, scale=float(inv_sqrt_d))
            # ---------------- phase 3b: P@V + denominators ----------------
            with (
                tc.tile_pool(name="obuf", bufs=2) as opool,
                tc.tile_pool(name="opsum", bufs=3, space="PSUM") as opsum,
                tc.tile_pool(name="dpsum", bufs=2, space="PSUM") as dpsum,
            ):
                for qt in range(NQT):
                    nkc = 8 * (qt + 1)
                    pso = [opsum.tile([128, 512], fp32, tag=f"po{dh}",
                                      name=f"pso{qt}_{dh}") for dh in range(2)]
                    denp = dpsum.tile([128, 1], fp32, tag="den",
                                      name=f"den{qt}")
                    for kc in range(nkc):
                        r, sl = kc // 4, kc % 4
                        vb = (r % 2) * 4096 + sl * 1024
                        vt = vfp[r // 2]
                        lhs = ptall[:, kc * 512 + qt * 128:
                                    kc * 512 + (qt + 1) * 128]
                        st = (kc == 0)
                        sp = (kc == nkc - 1)
                        # lhsT (P^T chunk) shared by the three rhs -> one
                        # stationary load serves d-half 0, d-half 1, denom
                        nc.tensor.matmul(pso[0][:], lhs,
                                         vt[:, vb:vb + 512],
                                         start=st, stop=sp)
                        nc.tensor.matmul(pso[1][:], lhs,
                                         vt[:, vb + 512:vb + 1024],
                                         start=st, stop=sp)
                        nc.tensor.matmul(denp[:], lhs, ones_sb[:],
                                         start=st, stop=sp)
                    recip = stpool.tile([128, 1], fp32, tag="recip")
                    nc.vector.reciprocal(recip[:], denp[:])
                    o_sb = opool.tile([128, D], fp32, tag="o")
                    # ship each d-half as soon as its scale completes
                    for dh in range(2):
                        nc.vector.tensor_scalar_mul(
                            o_sb[:, dh * 512:(dh + 1) * 512], pso[dh][:],
                            recip[:])
                        eng = nc.sync if dh == 0 else nc.scalar
                        eng.dma_start(
                            out[qt * 128:(qt + 1) * 128,
                                dh * 512:(dh + 1) * 512],
                            o_sb[:, dh * 512:(dh + 1) * 512])

    nc.compile()
    return nc


def _get_nc():
    if "nc" not in _CACHE:
        _CACHE["nc"] = _build()
    return _CACHE["nc"]


def make_in_maps(x, Wq, Wk, Wv):
    x_bf = np.ascontiguousarray(x).astype(BF16)
    wqt = np.ascontiguousarray(Wq.astype(BF16).T)
    wkt = np.ascontiguousarray(Wk.astype(BF16).T)
    wvt = np.ascontiguousarray(Wv.astype(BF16).T)
    in_maps = []
    j = np.arange(128)[:, None]                     # key-within-chunk
    u = np.arange(16)[None, :]                      # q - 16*kc
    for c in range(NC_N):
        # q row (global) = c + 8*(16*kc + u); key = 128*kc + j
        # keep iff c + 8*u >= j  (kc-independent)
        maskT = np.where(c + 8 * u >= j, 0.0, -30000.0).astype(np.float32)
        xt_q = np.ascontiguousarray(x_bf[c::NC_N].T)
        xt_kv = np.ascontiguousarray(x_bf[c * KVROWS:(c + 1) * KVROWS].T)
        in_maps.append({"xt_q": xt_q, "xt_kv": xt_kv, "wqt": wqt,
                        "wkt": wkt, "wvt": wvt, "maskT": maskT})
    return in_maps


def run(in_maps, trace=False, tmpdir=None, trace_cores=None):
    from concourse.bass_utils import run_bass_kernel_spmd
    nc = _get_nc()
    return run_bass_kernel_spmd(nc, in_maps, core_ids=list(range(NC_N)),
                                trace=trace, tmpdir=tmpdir,
                                trace_cores=trace_cores)


def kernel(x, Wq, Wk, Wv):
    res = run(make_in_maps(np.asarray(x), np.asarray(Wq),
                           np.asarray(Wk), np.asarray(Wv)))
    full = np.empty((S, D), np.float32)
    for c in range(NC_N):
        full[c::NC_N] = res.results[c]["out"]
    return full



# revision 2
# speedup vs baseline: 1.0445x; 1.0445x over previous
"""Causal attention (single head) on 8 Trainium2 NeuronCores.

Problem: x[4096,1024], Wq/Wk/Wv[1024,1024] (torch Linear layout, applied as
x @ W.T); out = renormalized-causal-softmax(Q K^T / 32) @ V, fp32, [4096,1024].

Distribution (hardcoded for S=4096, D=1024, 8 cores):
  - Q rows are sharded STRIDED: core c owns rows c::8, so every core has
    identical causal trip counts (SPMD: one program, data-only variation).
  - K/V rows are sharded CONTIGUOUS: core c computes K^T,V for rows
    [512c, 512c+512) in fp8e3 (e3m4: 4-bit mantissa; K/V values sit well
    inside its range, and it halves the exchange bytes vs bf16/e4m3).
  - Measured CC behavior on this runtime: the collective stream moves no
    bytes until ~75us after EXECUTION start (per-exec arming, independent
    of trigger time, op count, or payload), then streams at ~80GB/s wire
    with ~8us/op overhead.  Two collectives -- AG(KT, 2MB out) then
    AG(V, 4MB out) -- measured fastest; more splits lower the wire rate.
  - Attention is computed TRANSPOSED: S^T[k, q] = K^T-chunk.T @ Q^T with the
    128-key chunk stationary.  exp(S^T) IS P^T, exactly the lhsT layout the
    P@V matmuls need -- no PE transposes.  Causality: key chunk kc is needed
    only by q >= 16*kc (exact, uniformly over cores); within that only the
    first 16 q columns are partially masked, by ONE shared [128,16] additive
    -30000 pattern (keep iff c+8u >= j, core-dependent data).  The
    causally-dead strip [128*(kc//8), 16*kc) that P@V still touches is
    zeroed.  "softmax -> tril -> renormalize" == masked-exp / masked-sum,
    and scores/32 are within +-4 so exp needs no max-subtraction.
  - Denominators: P@V runs with P^T chunks stationary; a third 1-column
    matmul against ones rides on each stationary load and accumulates
    sum_k P^T[k, q] in PSUM -- the softmax denominator for free.
  - All matmuls bf16/fp8 mixed (fp32 matmul is 4x slower); PSUM accum fp32.
    (Measured dead end: filling the CC-wait PE gaps with dummy matmuls to
    keep the HAM clock gate warm made things WORSE -- the chip runs under a
    GPIO power cap, so extra PE work directly lowers the duty clock.)
"""

import numpy as np
import ml_dtypes

S, D, NC_N = 4096, 1024, 8
QROWS = S // NC_N            # 512 q rows per core
KVROWS = S // NC_N           # 512 kv rows per core
NQT = QROWS // 128           # 4 q-tiles of 128 rows per core
DC = D // 128                # 8 contraction chunks
NKC = S // 128               # 32 global key chunks
BF16 = ml_dtypes.bfloat16

_CACHE = {}


def _build():
    import concourse.bass as bass
    import concourse.mybir as mybir
    import concourse.tile as tile
    from concourse import bacc

    fp32 = mybir.dt.float32
    bf16 = mybir.dt.bfloat16
    fp8 = mybir.dt.float8e3

    nc = bacc.Bacc("TRN2", target_bir_lowering=False, debug=False,
                   num_devices=NC_N, enable_asserts=False)

    xt_q = nc.dram_tensor("xt_q", [D, QROWS], bf16, kind="ExternalInput").ap()
    xt_kv = nc.dram_tensor("xt_kv", [D, KVROWS], bf16, kind="ExternalInput").ap()
    wqt = nc.dram_tensor("wqt", [D, D], bf16, kind="ExternalInput").ap()
    wkt = nc.dram_tensor("wkt", [D, D], bf16, kind="ExternalInput").ap()
    wvt = nc.dram_tensor("wvt", [D, D], bf16, kind="ExternalInput").ap()
    maskT = nc.dram_tensor("maskT", [128, 16], fp32, kind="ExternalInput").ap()
    out = nc.dram_tensor("out", [QROWS, D], fp32, kind="ExternalOutput").ap()

    rg = [list(range(NC_N))]
    inv_sqrt_d = 1.0 / np.sqrt(np.float32(D))

    with tile.TileContext(nc) as tc:
        with (
            tc.tile_pool(name="dram", bufs=1, space="DRAM") as dram,
            tc.tile_pool(name="const", bufs=1) as cpool,
            tc.tile_pool(name="kvres", bufs=1) as kvpool,
            tc.tile_pool(name="stats", bufs=4) as stpool,
        ):
            kt_cc_in = dram.tile([D, KVROWS], fp8, name="kt_cc_in")
            v_cc_in = dram.tile([KVROWS, D], fp8, name="v_cc_in")
            kt_cc_out = dram.tile([NC_N, D, KVROWS], fp8, name="kt_cc_out",
                                  addr_space="Shared")
            v_cc_out = dram.tile([NC_N, KVROWS, D], fp8, name="v_cc_out",
                                 addr_space="Shared")

            ones_sb = cpool.tile([128, 1], bf16, name="ones_sb")
            nc.any.memset(ones_sb[:], 1.0)
            mask_sb = cpool.tile([128, 16], fp32, name="mask_sb")
            nc.scalar.dma_start(mask_sb[:], maskT[:])
            wrm = cpool.tile([128, 512], bf16, name="wrm")
            nc.vector.memset(wrm[:], 0.0)

            # gathered K^T, 2 shards per tile:
            #   ktq[h][p, (r-2h)*4096 + dc*512 + j] = K[512r+j, 128dc+p]
            ktq = [kvpool.tile([128, 2 * DC * 512], fp8, name=f"ktq{h}")
                   for h in range(4)]
            # gathered V, 2 shards per tile:
            #   vfp[g][p, (r-2g)*4096 + sl*1024 + dh*512 + j]
            #     = V[512r+128sl+p, 512dh+j]
            vfp = [kvpool.tile([128, 2 * 4096], fp8, name=f"vfp{g}")
                   for g in range(4)]
            # Q^T resident: qt_sb[p, dc*512 + q] = Q[q, 128dc+p]
            qt_sb = kvpool.tile([128, DC * 512], bf16, name="qt_sb")
            # P^T resident: ptall[p, kc*512 + q] = P[q, 128kc+p]
            ptall = kvpool.tile([128, NKC * 512], bf16, name="ptall")

            # ---------------- phase 1: projections + gathers ----------------
            with (
                tc.tile_pool(name="wpool", bufs=12) as wpool,
                tc.tile_pool(name="xpool", bufs=1) as xpool,
                tc.tile_pool(name="loc", bufs=4) as locpool,
                tc.tile_pool(name="ppsum", bufs=3, space="PSUM") as ppsum,
            ):
                # PE warmup: the HAM clock gate holds the PE at 1.2 GHz until
                # it has been busy ~3.4us; burn dummy matmuls during the
                # initial weight DMA so the projections run warm (measured
                # better than omitting it, despite the GPIO power cap).
                wps = ppsum.tile([128, 512], fp32, tag="warm")
                for i in range(24):
                    nc.tensor.matmul(wps[:], wrm[:, 0:128], wrm[:],
                                     start=(i == 0), stop=(i == 23))

                # K-projection inputs first (critical path to the CC stream)
                wk, xkv = [], []
                for dc in range(DC):
                    tw = wpool.tile([128, D], bf16, name=f"wk{dc}", tag="w")
                    nc.scalar.dma_start(tw[:], wkt[dc * 128:(dc + 1) * 128, :])
                    wk.append(tw)
                    tx = xpool.tile([128, KVROWS], bf16, name=f"xkv{dc}")
                    nc.scalar.dma_start(tx[:], xt_kv[dc * 128:(dc + 1) * 128, :])
                    xkv.append(tx)

                # K^T_local[d, s] = (Wk @ x_kv^T): lhsT = Wk^T chunk, rhs = x_kv^T
                for po in range(DC):
                    ps = ppsum.tile([128, 512], fp32, tag="pp")
                    for dc in range(DC):
                        nc.tensor.matmul(ps[:], wk[dc][:, po * 128:(po + 1) * 128],
                                         xkv[dc][:],
                                         start=(dc == 0), stop=(dc == DC - 1))
                    loc = locpool.tile([128, 512], fp8, tag="lock")
                    nc.vector.tensor_copy(loc[:], ps[:])
                    nc.sync.dma_start(kt_cc_in[po * 128:(po + 1) * 128, :], loc[:])

                nc.gpsimd.collective_compute(
                    "AllGather", mybir.AluOpType.bypass, replica_groups=rg,
                    ins=[kt_cc_in[:]], outs=[kt_cc_out[:]])

                # V_local[s, d] = x_kv @ Wv^T: lhsT = x_kv^T chunk, rhs = Wv^T
                wv = []
                for dc in range(DC):
                    tw = wpool.tile([128, D], bf16, name=f"wv{dc}", tag="w")
                    nc.scalar.dma_start(tw[:], wvt[dc * 128:(dc + 1) * 128, :])
                    wv.append(tw)
                for st in range(4):
                    for dh in range(2):
                        ps = ppsum.tile([128, 512], fp32, tag="pp")
                        for dc in range(DC):
                            nc.tensor.matmul(
                                ps[:], xkv[dc][:, st * 128:(st + 1) * 128],
                                wv[dc][:, dh * 512:(dh + 1) * 512],
                                start=(dc == 0), stop=(dc == DC - 1))
                        loc = locpool.tile([128, 512], fp8, tag="locv")
                        nc.vector.tensor_copy(loc[:], ps[:])
                        nc.sync.dma_start(
                            v_cc_in[st * 128:(st + 1) * 128,
                                    dh * 512:(dh + 1) * 512], loc[:])
                nc.gpsimd.collective_compute(
                    "AllGather", mybir.AluOpType.bypass, replica_groups=rg,
                    ins=[v_cc_in[:]], outs=[v_cc_out[:]])

                # Q^T[d, q]: lhsT = Wq^T chunk, rhs = x_q^T  -> straight to SBUF
                wq, xq = [], []
                for dc in range(DC):
                    tw = wpool.tile([128, D], bf16, name=f"wq{dc}", tag="w")
                    nc.scalar.dma_start(tw[:], wqt[dc * 128:(dc + 1) * 128, :])
                    wq.append(tw)
                    tx = xpool.tile([128, QROWS], bf16, name=f"xq{dc}")
                    nc.scalar.dma_start(tx[:], xt_q[dc * 128:(dc + 1) * 128, :])
                    xq.append(tx)
                for po in range(DC):
                    ps = ppsum.tile([128, 512], fp32, tag="pp")
                    for dc in range(DC):
                        nc.tensor.matmul(ps[:], wq[dc][:, po * 128:(po + 1) * 128],
                                         xq[dc][:],
                                         start=(dc == 0), stop=(dc == DC - 1))
                    nc.vector.tensor_copy(qt_sb[:, po * 512:(po + 1) * 512], ps[:])

            # ---------------- phase 2: pull gathered K/V into SBUF ----------
            # Pulls are issued as 512KB per-shard halves in CONSUMPTION
            # order, alternating the two queues.  With the old 4x1MB
            # concurrent pulls the bandwidth was fair-shared, so ktq[0]
            # (the only tile S^T's first 8 chunks need -- Tile tracks
            # subregion deps) landed ~17us after the AllGather; ordered
            # halves land it in ~4-5us and S^T starts that much earlier.
            # Consumption (3.5us/shard) then outpaces each 4.3us half-pull
            # only briefly.  Same for V: vfp[0] (q-tile 0's shards) first.
            for h in range(4):
                for r2 in range(2):
                    eng = nc.sync if (2 * h + r2) % 2 == 0 else nc.scalar
                    eng.dma_start(
                        ktq[h][:, r2 * 4096:(r2 + 1) * 4096].rearrange(
                            "p (a j) -> p a j", a=DC),
                        kt_cc_out[2 * h + r2].rearrange(
                            "(a p) j -> p a j", p=128))
            for g in range(4):
                for r2 in range(2):
                    eng = nc.sync if (2 * g + r2) % 2 == 0 else nc.scalar
                    eng.dma_start(
                        vfp[g][:, r2 * 4096:(r2 + 1) * 4096].rearrange(
                            "p (s q) -> p s q", s=4),
                        v_cc_out[2 * g + r2].rearrange(
                            "(s p) q -> p s q", p=128))

            # ---------------- phase 3a: S^T + exp -> P^T ----------------
            with tc.tile_pool(name="spsum", bufs=6, space="PSUM") as spsum:
                for kc in range(NKC):
                    qt0 = kc // 8
                    qoff = 16 * kc
                    w = 512 - qoff
                    r, kci = kc // 4, kc % 4
                    if qoff > 128 * qt0:
                        nc.vector.memset(
                            ptall[:, kc * 512 + 128 * qt0:kc * 512 + qoff], 0.0)
                    psT = spsum.tile([128, 512], fp32, tag="s")
                    rb = (r % 2) * 4096
                    for dc in range(DC):
                        nc.tensor.matmul(
                            psT[:, :w],
                            ktq[r // 2][:, rb + dc * 512 + kci * 128:
                                        rb + dc * 512 + (kci + 1) * 128],
                            qt_sb[:, dc * 512 + qoff:(dc + 1) * 512],
                            start=(dc == 0), stop=(dc == DC - 1))
                    nc.vector.tensor_add(psT[:, 0:16], psT[:, 0:16],
                                         mask_sb[:])
                    nc.scalar.activation(
                        ptall[:, kc * 512 + qoff:(kc + 1) * 512], psT[:, :w],
                        mybir.ActivationFunctionType.Exp,
                        bias=0.0, scale=float(inv_sqrt_d))
            # ---------------- phase 3b: P@V + denominators ----------------
            with (
                tc.tile_pool(name="obuf", bufs=2) as opool,
                tc.tile_pool(name="opsum", bufs=3, space="PSUM") as opsum,
                tc.tile_pool(name="dpsum", bufs=2, space="PSUM") as dpsum,
            ):
                for qt in range(NQT):
                    nkc = 8 * (qt + 1)
                    pso = [opsum.tile([128, 512], fp32, tag=f"po{dh}",
                                      name=f"pso{qt}_{dh}") for dh in range(2)]
                    denp = dpsum.tile([128, 1], fp32, tag="den",
                                      name=f"den{qt}")
                    for kc in range(nkc):
                        r, sl = kc // 4, kc % 4
                        vb = (r % 2) * 4096 + sl * 1024
                        vt = vfp[r // 2]
                        lhs = ptall[:, kc * 512 + qt * 128:
                                    kc * 512 + (qt + 1) * 128]
                        st = (kc == 0)
                        sp = (kc == nkc - 1)
                        # lhsT (P^T chunk) shared by the three rhs -> one
                        # stationary load serves d-half 0, d-half 1, denom
                        nc.tensor.matmul(pso[0][:], lhs,
                                         vt[:, vb:vb + 512],
                                         start=st, stop=sp)
                        nc.tensor.matmul(pso[1][:], lhs,
                                         vt[:, vb + 512:vb + 1024],
                                         start=st, stop=sp)
                        nc.tensor.matmul(denp[:], lhs, ones_sb[:],
                                         start=st, stop=sp)
                    recip = stpool.tile([128, 1], fp32, tag="recip")
                    nc.vector.reciprocal(recip[:], denp[:])
                    o_sb = opool.tile([128, D], fp32, tag="o")
                    # ship each d-half as soon as its scale completes
                    for dh in range(2):
                        nc.vector.tensor_scalar_mul(
                            o_sb[:, dh * 512:(dh + 1) * 512], pso[dh][:],
                            recip[:])
                        eng = nc.sync if dh == 0 else nc.scalar
                        eng.dma_start(
                            out[qt * 128:(qt + 1) * 128,
                                dh * 512:(dh + 1) * 512],
                            o_sb[:, dh * 512:(dh + 1) * 512])

    nc.compile()
    return nc


def _get_nc():
    if "nc" not in _CACHE:
        _CACHE["nc"] = _build()
    return _CACHE["nc"]


def make_in_maps(x, Wq, Wk, Wv):
    x_bf = np.ascontiguousarray(x).astype(BF16)
    wqt = np.ascontiguousarray(Wq.astype(BF16).T)
    wkt = np.ascontiguousarray(Wk.astype(BF16).T)
    wvt = np.ascontiguousarray(Wv.astype(BF16).T)
    in_maps = []
    j = np.arange(128)[:, None]                     # key-within-chunk
    u = np.arange(16)[None, :]                      # q - 16*kc
    for c in range(NC_N):
        # q row (global) = c + 8*(16*kc + u); key = 128*kc + j
        # keep iff c + 8*u >= j  (kc-independent)
        maskT = np.where(c + 8 * u >= j, 0.0, -30000.0).astype(np.float32)
        xt_q = np.ascontiguousarray(x_bf[c::NC_N].T)
        xt_kv = np.ascontiguousarray(x_bf[c * KVROWS:(c + 1) * KVROWS].T)
        in_maps.append({"xt_q": xt_q, "xt_kv": xt_kv, "wqt": wqt,
                        "wkt": wkt, "wvt": wvt, "maskT": maskT})
    return in_maps


def run(in_maps, trace=False, tmpdir=None, trace_cores=None):
    from concourse.bass_utils import run_bass_kernel_spmd
    nc = _get_nc()
    return run_bass_kernel_spmd(nc, in_maps, core_ids=list(range(NC_N)),
                                trace=trace, tmpdir=tmpdir,
                                trace_cores=trace_cores)


def kernel(x, Wq, Wk, Wv):
    res = run(make_in_maps(np.asarray(x), np.asarray(Wq),
                           np.asarray(Wk), np.asarray(Wv)))
    full = np.empty((S, D), np.float32)
    for c in range(NC_N):
        full[c::NC_N] = res.results[c]["out"]
    return full



# revision 3
# speedup vs baseline: 1.0921x; 1.0455x over previous
"""Causal attention (single head) on 8 Trainium2 NeuronCores.

Problem: x[4096,1024], Wq/Wk/Wv[1024,1024] (torch Linear layout, applied as
x @ W.T); out = renormalized-causal-softmax(Q K^T / 32) @ V, fp32, [4096,1024].

Distribution (hardcoded for S=4096, D=1024, 8 cores):
  - Q rows are sharded STRIDED: core c owns rows c::8, so every core has
    identical causal trip counts (SPMD: one program, data-only variation).
  - K/V rows are sharded CONTIGUOUS: core c computes K^T,V for rows
    [512c, 512c+512) in fp8e3 (e3m4: 4-bit mantissa; K/V values sit well
    inside its range, and it halves the exchange bytes vs bf16/e4m3).
  - Measured CC behavior on this runtime: the collective stream moves no
    bytes until ~75us after EXECUTION start (per-exec arming, independent
    of trigger time, op count, or payload), then streams at ~80GB/s wire
    with ~8us/op overhead.  Two collectives -- AG(KT, 2MB out) then
    AG(V, 4MB out) -- measured fastest; more splits lower the wire rate.
  - Attention is computed TRANSPOSED: S^T[k, q] = K^T-chunk.T @ Q^T with the
    128-key chunk stationary.  exp(S^T) IS P^T, exactly the lhsT layout the
    P@V matmuls need -- no PE transposes.  Causality: key chunk kc is needed
    only by q >= 16*kc (exact, uniformly over cores); within that only the
    first 16 q columns are partially masked, by ONE shared [128,16] additive
    -30000 pattern (keep iff c+8u >= j, core-dependent data).  The
    causally-dead strip [128*(kc//8), 16*kc) that P@V still touches is
    zeroed.  "softmax -> tril -> renormalize" == masked-exp / masked-sum,
    and scores/32 are within +-4 so exp needs no max-subtraction.
  - Denominators: P@V runs with P^T chunks stationary; a third 1-column
    matmul against ones rides on each stationary load and accumulates
    sum_k P^T[k, q] in PSUM -- the softmax denominator for free.
  - All matmuls bf16/fp8 mixed (fp32 matmul is 4x slower); PSUM accum fp32.
    (Measured dead end: filling the CC-wait PE gaps with dummy matmuls to
    keep the HAM clock gate warm made things WORSE -- the chip runs under a
    GPIO power cap, so extra PE work directly lowers the duty clock.)
"""

import numpy as np
import ml_dtypes

S, D, NC_N = 4096, 1024, 8
QROWS = S // NC_N            # 512 q rows per core
KVROWS = S // NC_N           # 512 kv rows per core
NQT = QROWS // 128           # 4 q-tiles of 128 rows per core
DC = D // 128                # 8 contraction chunks
NKC = S // 128               # 32 global key chunks
BF16 = ml_dtypes.bfloat16

_CACHE = {}


def _build():
    import concourse.bass as bass
    import concourse.mybir as mybir
    import concourse.tile as tile
    from concourse import bacc

    fp32 = mybir.dt.float32
    bf16 = mybir.dt.bfloat16
    fp8 = mybir.dt.float8e3

    nc = bacc.Bacc("TRN2", target_bir_lowering=False, debug=False,
                   num_devices=NC_N, enable_asserts=False)

    xt_q = nc.dram_tensor("xt_q", [D, QROWS], bf16, kind="ExternalInput").ap()
    xt_kv = nc.dram_tensor("xt_kv", [D, KVROWS], bf16, kind="ExternalInput").ap()
    wqt = nc.dram_tensor("wqt", [D, D], bf16, kind="ExternalInput").ap()
    wkt = nc.dram_tensor("wkt", [D, D], bf16, kind="ExternalInput").ap()
    wvt = nc.dram_tensor("wvt", [D, D], bf16, kind="ExternalInput").ap()
    maskT = nc.dram_tensor("maskT", [128, 16], fp32, kind="ExternalInput").ap()
    out = nc.dram_tensor("out", [QROWS, D], fp32, kind="ExternalOutput").ap()

    rg = [list(range(NC_N))]
    inv_sqrt_d = 1.0 / np.sqrt(np.float32(D))

    with tile.TileContext(nc) as tc:
        with (
            tc.tile_pool(name="dram", bufs=1, space="DRAM") as dram,
            tc.tile_pool(name="const", bufs=1) as cpool,
            tc.tile_pool(name="kvres", bufs=1) as kvpool,
            tc.tile_pool(name="stats", bufs=4) as stpool,
        ):
            kt_cc_in = dram.tile([D, KVROWS], fp8, name="kt_cc_in")
            v_cc_in = dram.tile([KVROWS, D], fp8, name="v_cc_in")
            kt_cc_out = dram.tile([NC_N, D, KVROWS], fp8, name="kt_cc_out",
                                  addr_space="Shared")
            v_cc_out = dram.tile([NC_N, KVROWS, D], fp8, name="v_cc_out",
                                 addr_space="Shared")

            ones_sb = cpool.tile([128, 1], bf16, name="ones_sb")
            nc.any.memset(ones_sb[:], 1.0)
            mask_sb = cpool.tile([128, 16], fp32, name="mask_sb")
            nc.scalar.dma_start(mask_sb[:], maskT[:])
            wrm = cpool.tile([128, 512], bf16, name="wrm")
            nc.vector.memset(wrm[:], 0.0)

            # gathered K^T, 2 shards per tile:
            #   ktq[h][p, (r-2h)*4096 + dc*512 + j] = K[512r+j, 128dc+p]
            ktq = [kvpool.tile([128, 2 * DC * 512], fp8, name=f"ktq{h}")
                   for h in range(4)]
            # gathered V, 2 shards per tile:
            #   vfp[g][p, (r-2g)*4096 + sl*1024 + dh*512 + j]
            #     = V[512r+128sl+p, 512dh+j]
            vfp = [kvpool.tile([128, 2 * 4096], fp8, name=f"vfp{g}")
                   for g in range(4)]
            # Q^T resident: qt_sb[p, dc*512 + q] = Q[q, 128dc+p]
            qt_sb = kvpool.tile([128, DC * 512], bf16, name="qt_sb")
            # P^T resident: ptall[p, kc*512 + q] = P[q, 128kc+p]
            ptall = kvpool.tile([128, NKC * 512], bf16, name="ptall")

            # ---------------- phase 1: projections + gathers ----------------
            with (
                tc.tile_pool(name="wpool", bufs=12) as wpool,
                tc.tile_pool(name="xpool", bufs=1) as xpool,
                tc.tile_pool(name="loc", bufs=4) as locpool,
                tc.tile_pool(name="ppsum", bufs=3, space="PSUM") as ppsum,
            ):
                # PE warmup: the HAM clock gate holds the PE at 1.2 GHz until
                # it has been busy ~3.4us; burn dummy matmuls during the
                # initial weight DMA so the projections run warm (measured
                # better than omitting it, despite the GPIO power cap).
                wps = ppsum.tile([128, 512], fp32, tag="warm")
                for i in range(24):
                    nc.tensor.matmul(wps[:], wrm[:, 0:128], wrm[:],
                                     start=(i == 0), stop=(i == 23))

                # K-projection inputs first (critical path to the CC stream)
                wk, xkv = [], []
                for dc in range(DC):
                    tw = wpool.tile([128, D], bf16, name=f"wk{dc}", tag="w")
                    nc.scalar.dma_start(tw[:], wkt[dc * 128:(dc + 1) * 128, :])
                    wk.append(tw)
                    tx = xpool.tile([128, KVROWS], bf16, name=f"xkv{dc}")
                    nc.scalar.dma_start(tx[:], xt_kv[dc * 128:(dc + 1) * 128, :])
                    xkv.append(tx)

                # K^T_local[d, s] = (Wk @ x_kv^T): lhsT = Wk^T chunk, rhs = x_kv^T
                for po in range(DC):
                    ps = ppsum.tile([128, 512], fp32, tag="pp")
                    for dc in range(DC):
                        nc.tensor.matmul(ps[:], wk[dc][:, po * 128:(po + 1) * 128],
                                         xkv[dc][:],
                                         start=(dc == 0), stop=(dc == DC - 1))
                    loc = locpool.tile([128, 512], fp8, tag="lock")
                    nc.vector.tensor_copy(loc[:], ps[:])
                    nc.sync.dma_start(kt_cc_in[po * 128:(po + 1) * 128, :], loc[:])

                nc.gpsimd.collective_compute(
                    "AllGather", mybir.AluOpType.bypass, replica_groups=rg,
                    ins=[kt_cc_in[:]], outs=[kt_cc_out[:]])

                # V_local[s, d] = x_kv @ Wv^T: lhsT = x_kv^T chunk, rhs = Wv^T
                wv = []
                for dc in range(DC):
                    tw = wpool.tile([128, D], bf16, name=f"wv{dc}", tag="w")
                    nc.scalar.dma_start(tw[:], wvt[dc * 128:(dc + 1) * 128, :])
                    wv.append(tw)
                for st in range(4):
                    for dh in range(2):
                        ps = ppsum.tile([128, 512], fp32, tag="pp")
                        for dc in range(DC):
                            nc.tensor.matmul(
                                ps[:], xkv[dc][:, st * 128:(st + 1) * 128],
                                wv[dc][:, dh * 512:(dh + 1) * 512],
                                start=(dc == 0), stop=(dc == DC - 1))
                        loc = locpool.tile([128, 512], fp8, tag="locv")
                        nc.vector.tensor_copy(loc[:], ps[:])
                        nc.sync.dma_start(
                            v_cc_in[st * 128:(st + 1) * 128,
                                    dh * 512:(dh + 1) * 512], loc[:])
                nc.gpsimd.collective_compute(
                    "AllGather", mybir.AluOpType.bypass, replica_groups=rg,
                    ins=[v_cc_in[:]], outs=[v_cc_out[:]])

                # Q^T[d, q]: lhsT = Wq^T chunk, rhs = x_q^T  -> straight to SBUF
                wq, xq = [], []
                for dc in range(DC):
                    tw = wpool.tile([128, D], bf16, name=f"wq{dc}", tag="w")
                    nc.scalar.dma_start(tw[:], wqt[dc * 128:(dc + 1) * 128, :])
                    wq.append(tw)
                    tx = xpool.tile([128, QROWS], bf16, name=f"xq{dc}")
                    nc.scalar.dma_start(tx[:], xt_q[dc * 128:(dc + 1) * 128, :])
                    xq.append(tx)
                for po in range(DC):
                    ps = ppsum.tile([128, 512], fp32, tag="pp")
                    for dc in range(DC):
                        nc.tensor.matmul(ps[:], wq[dc][:, po * 128:(po + 1) * 128],
                                         xq[dc][:],
                                         start=(dc == 0), stop=(dc == DC - 1))
                    nc.vector.tensor_copy(qt_sb[:, po * 512:(po + 1) * 512], ps[:])

            # ---------------- phase 2: pull gathered K/V into SBUF ----------
            # Pulls are issued as 512KB per-shard halves in CONSUMPTION
            # order, alternating the two queues.  With the old 4x1MB
            # concurrent pulls the bandwidth was fair-shared, so ktq[0]
            # (the only tile S^T's first 8 chunks need -- Tile tracks
            # subregion deps) landed ~17us after the AllGather; ordered
            # halves land it in ~4-5us and S^T starts that much earlier.
            # Consumption (3.5us/shard) then outpaces each 4.3us half-pull
            # only briefly.  Same for V: vfp[0] (q-tile 0's shards) first.
            for h in range(4):
                for r2 in range(2):
                    eng = nc.sync if (2 * h + r2) % 2 == 0 else nc.scalar
                    eng.dma_start(
                        ktq[h][:, r2 * 4096:(r2 + 1) * 4096].rearrange(
                            "p (a j) -> p a j", a=DC),
                        kt_cc_out[2 * h + r2].rearrange(
                            "(a p) j -> p a j", p=128))
            for g in range(4):
                for r2 in range(2):
                    eng = nc.sync if (2 * g + r2) % 2 == 0 else nc.scalar
                    eng.dma_start(
                        vfp[g][:, r2 * 4096:(r2 + 1) * 4096].rearrange(
                            "p (s q) -> p s q", s=4),
                        v_cc_out[2 * g + r2].rearrange(
                            "(s p) q -> p s q", p=128))

            # ---------------- phase 3a: S^T + exp -> P^T ----------------
            with tc.tile_pool(name="spsum", bufs=6, space="PSUM") as spsum:
                for kc in range(NKC):
                    qt0 = kc // 8
                    qoff = 16 * kc
                    w = 512 - qoff
                    r, kci = kc // 4, kc % 4
                    if qoff > 128 * qt0:
                        nc.vector.memset(
                            ptall[:, kc * 512 + 128 * qt0:kc * 512 + qoff], 0.0)
                    psT = spsum.tile([128, 512], fp32, tag="s")
                    rb = (r % 2) * 4096
                    for dc in range(DC):
                        nc.tensor.matmul(
                            psT[:, :w],
                            ktq[r // 2][:, rb + dc * 512 + kci * 128:
                                        rb + dc * 512 + (kci + 1) * 128],
                            qt_sb[:, dc * 512 + qoff:(dc + 1) * 512],
                            start=(dc == 0), stop=(dc == DC - 1))
                    nc.vector.tensor_add(psT[:, 0:16], psT[:, 0:16],
                                         mask_sb[:])
                    nc.scalar.activation(
                        ptall[:, kc * 512 + qoff:(kc + 1) * 512], psT[:, :w],
                        mybir.ActivationFunctionType.Exp,
                        bias=0.0, scale=float(inv_sqrt_d))
            # ---------------- phase 3b: P@V + denominators ----------------
            with (
                tc.tile_pool(name="obuf", bufs=2) as opool,
                tc.tile_pool(name="opsum", bufs=3, space="PSUM") as opsum,
                tc.tile_pool(name="dpsum", bufs=2, space="PSUM") as dpsum,
            ):
                for qt in range(NQT):
                    nkc = 8 * (qt + 1)
                    pso = [opsum.tile([128, 512], fp32, tag=f"po{dh}",
                                      name=f"pso{qt}_{dh}") for dh in range(2)]
                    denp = dpsum.tile([128, 1], fp32, tag="den",
                                      name=f"den{qt}")
                    for kc in range(nkc):
                        r, sl = kc // 4, kc % 4
                        vb = (r % 2) * 4096 + sl * 1024
                        vt = vfp[r // 2]
                        lhs = ptall[:, kc * 512 + qt * 128:
                                    kc * 512 + (qt + 1) * 128]
                        st = (kc == 0)
                        sp = (kc == nkc - 1)
                        # lhsT (P^T chunk) shared by the three rhs -> one
                        # stationary load serves d-half 0, d-half 1, denom
                        nc.tensor.matmul(pso[0][:], lhs,
                                         vt[:, vb:vb + 512],
                                         start=st, stop=sp)
                        nc.tensor.matmul(pso[1][:], lhs,
                                         vt[:, vb + 512:vb + 1024],
                                         start=st, stop=sp)
                        nc.tensor.matmul(denp[:], lhs, ones_sb[:],
                                         start=st, stop=sp)
                    recip = stpool.tile([128, 1], fp32, tag="recip")
                    nc.vector.reciprocal(recip[:], denp[:])
                    o_sb = opool.tile([128, D], fp32, tag="o")
                    # quarter-granularity scale+ship on alternating queues:
                    # halves the exposed scale->DMA chain on the final tile
                    for dq in range(4):
                        lo, hi = dq * 256, (dq + 1) * 256
                        nc.vector.tensor_scalar_mul(
                            o_sb[:, lo:hi], pso[dq // 2][:, lo % 512:
                                                         lo % 512 + 256],
                            recip[:])
                        eng = nc.sync if dq % 2 == 0 else nc.scalar
                        eng.dma_start(
                            out[qt * 128:(qt + 1) * 128, lo:hi],
                            o_sb[:, lo:hi])

    nc.compile()
    return nc


def _get_nc():
    if "nc" not in _CACHE:
        _CACHE["nc"] = _build()
    return _CACHE["nc"]


def make_in_maps(x, Wq, Wk, Wv):
    x_bf = np.ascontiguousarray(x).astype(BF16)
    wqt = np.ascontiguousarray(Wq.astype(BF16).T)
    wkt = np.ascontiguousarray(Wk.astype(BF16).T)
    wvt = np.ascontiguousarray(Wv.astype(BF16).T)
    in_maps = []
    j = np.arange(128)[:, None]                     # key-within-chunk
    u = np.arange(16)[None, :]                      # q - 16*kc
    for c in range(NC_N):
        # q row (global) = c + 8*(16*kc + u); key = 128*kc + j
        # keep iff c + 8*u >= j  (kc-independent)
        maskT = np.where(c + 8 * u >= j, 0.0, -30000.0).astype(np.float32)
        xt_q = np.ascontiguousarray(x_bf[c::NC_N].T)
        xt_kv = np.ascontiguousarray(x_bf[c * KVROWS:(c + 1) * KVROWS].T)
        in_maps.append({"xt_q": xt_q, "xt_kv": xt_kv, "wqt": wqt,
                        "wkt": wkt, "wvt": wvt, "maskT": maskT})
    return in_maps


def run(in_maps, trace=False, tmpdir=None, trace_cores=None):
    from concourse.bass_utils import run_bass_kernel_spmd
    nc = _get_nc()
    return run_bass_kernel_spmd(nc, in_maps, core_ids=list(range(NC_N)),
                                trace=trace, tmpdir=tmpdir,
                                trace_cores=trace_cores)


def kernel(x, Wq, Wk, Wv):
    res = run(make_in_maps(np.asarray(x), np.asarray(Wq),
                           np.asarray(Wk), np.asarray(Wv)))
    full = np.empty((S, D), np.float32)
    for c in range(NC_N):
        full[c::NC_N] = res.results[c]["out"]
    return full

